# revision 1
# baseline (speedup 1.0000x reference)
"""Causal multi-head self-attention (B=4, S=2048, D=1024, H=16, RoPE) on 8
Trainium2 NeuronCores.

Sharding (hardcoded): core c handles batch b = c//2 and head group g = c%2
(8 of the 16 heads).  Data parallel over B, tensor parallel over heads for
the Wq/Wk/Wv projections and over Wo rows/columns: each core computes the
attention output for its 8 heads, the two cores of a pair AllGather their
(normalized) per-head-pair attention chunks in 512-column slices, and each
core then computes a disjoint 512-wide column slice of the final Wo
projection for its batch, so the host only concatenates slices (no
host-side arithmetic).

Compute is bf16 on the tensor engine (f32 PSUM accumulation) throughout.

Per head pair (hp = 2 heads = 128 q/k/v dims):
  1. qT/kT/vT = W.T @ xT as [128 e, S] tiles straight off the tensor engine
     (x and weights fed pre-transposed from the host).
  2. RoPE on the vector engine: the PSUM result is evacuated to SBUF bf16
     first (frees the PSUM slot after one pass), then rotated with
     host-built cos/sin tables.  Wq/Wk rows are host-permuted so each
     rotation partner sits 16 partitions away within a 32-partition group,
     which a single stream_shuffle realizes.
  3. vT is transposed back to v [sk, d] blocks on the tensor engine and
     augmented with a ones column so the PV matmul also produces the
     softmax denominator (PSUM row 64).
  4. Flash-style causal attention over [sk 128, sq 512] score blocks held
     transposed; the two heads' blocks land in adjacent PSUM banks so exp
     (scalar engine, scale 1/8, no max subtraction -- scores are O(1) by
     construction) and the causal mask multiply run as [128, 1024] ops.
  5. Per sq-slice: PV outputs staged unnormalized to SBUF, denominators
     stacked at partitions {0,32} of a staging tile so one reciprocal
     serves both heads, normalization via gpsimd partition-broadcast
     (full-tile only -- HW ignores AP offsets), then a per-slice AllGather
     with the paired core.  Wo partials for gathered slices are emitted
     interleaved into later attention steps as tensor-engine filler.
"""

import numpy as np

D_MODEL = 1024
NUM_HEADS = 16
ROPE_THETA = 10000.0
DH = D_MODEL // NUM_HEADS  # 64
SQT = 512  # sq tile width (= PSUM bank width in f32)


# ---------------------------------------------------------------------------
# Device kernel builder
# ---------------------------------------------------------------------------

def build_kernel(n_cores: int = 8, S: int = 2048):
    import concourse.bass as bass
    import concourse.mybir as mybir
    import concourse.tile as tile
    from concourse import bacc
    from concourse.masks import make_identity

    F32 = mybir.dt.float32
    BF16 = mybir.dt.bfloat16
    Exp = mybir.ActivationFunctionType.Exp
    mult = mybir.AluOpType.mult
    add = mybir.AluOpType.add

    D = D_MODEL
    NC = D // 128          # 8 d-chunks
    NSB = S // 128         # s 128-blocks
    NSQ = S // SQT         # sq 512-tiles
    NHP = 4                # head pairs per core
    SWAP16 = list(range(16, 32)) + list(range(16))

    nc = bacc.Bacc("TRN2", target_bir_lowering=False, debug=False,
                   num_devices=n_cores)

    xT = nc.dram_tensor("xT", [128, NC, S], BF16, kind="ExternalInput")
    wqT = nc.dram_tensor("wqT", [128, NC, NHP, 128], BF16, kind="ExternalInput")
    wkT = nc.dram_tensor("wkT", [128, NC, NHP, 128], BF16, kind="ExternalInput")
    wvT = nc.dram_tensor("wvT", [128, NC, NHP, 128], BF16, kind="ExternalInput")
    woT = nc.dram_tensor("woT", [128, NC, SQT], BF16, kind="ExternalInput")
    cosT = nc.dram_tensor("cosT", [128, S], BF16, kind="ExternalInput")
    sinT = nc.dram_tensor("sinT", [128, S], BF16, kind="ExternalInput")
    masks = nc.dram_tensor("masks", [128, 4, 2 * SQT], BF16, kind="ExternalInput")
    out = nc.dram_tensor("out", [S, SQT], F32, kind="ExternalOutput")

    groups = [[2 * i, 2 * i + 1] for i in range(n_cores // 2)]

    with tile.TileContext(nc) as tc:
        with (
            tc.tile_pool(name="const", bufs=1) as constp,
            tc.tile_pool(name="w", bufs=2) as wp,
            tc.tile_pool(name="qk", bufs=2) as qkp,
            tc.tile_pool(name="v", bufs=2) as vp,
            tc.tile_pool(name="probs", bufs=6) as probsp,
            tc.tile_pool(name="rope", bufs=2) as ropep,
            tc.tile_pool(name="attn", bufs=2) as attnp,
            tc.tile_pool(name="ag", bufs=4) as agp,
            tc.tile_pool(name="acc", bufs=1) as accp,
            tc.tile_pool(name="small", bufs=3) as smallp,
            tc.tile_pool(name="norm", bufs=3) as normp,
            tc.tile_pool(name="unn", bufs=1) as unnp,
            tc.tile_pool(name="psA", bufs=2, space="PSUM") as psA,
            tc.tile_pool(name="psQK", bufs=2, space="PSUM") as psQK,
            tc.tile_pool(name="psPV", bufs=2, space="PSUM") as psPV,
            tc.tile_pool(name="dram", bufs=4, space="DRAM") as dramp,
        ):
            # --- one-time loads -------------------------------------------
            xt_sb = constp.tile([128, NC, S], BF16, tag="xt")
            for c in range(NC):
                nc.sync.dma_start(xt_sb[:, c, :], xT[:, c, :])
            wo_sb = constp.tile([128, NC, SQT], BF16, tag="wo")
            nc.sync.dma_start(wo_sb[:], woT[:])
            cos_sb = constp.tile([128, S], BF16, tag="cos")
            nc.sync.dma_start(cos_sb[:], cosT[:])
            sin_sb = constp.tile([128, S], BF16, tag="sin")
            nc.sync.dma_start(sin_sb[:], sinT[:])
            mask_sb = constp.tile([128, 4, 2 * SQT], BF16, tag="mask")
            nc.sync.dma_start(mask_sb[:], masks[:])
            ident = constp.tile([128, 128], BF16, tag="ident")
            make_identity(nc, ident[:])

            out_acc = accp.tile([128, NSB, SQT], F32, tag="oacc")

            def emit_wo_chunk(hp, j, ag0c, ag1c):
                # Wo partial for s rows [512j, 512j+512) of head pair hp.
                final = hp == NHP - 1
                for t in range(SQT // 128):
                    sb = (SQT // 128) * j + t
                    tsl = bass.ts(t, 128)
                    ps = psA.tile([128, SQT], F32, tag="psA")
                    nc.tensor.matmul(ps[:], ag0c[:, tsl], wo_sb[:, hp, :],
                                     start=True, stop=False)
                    nc.tensor.matmul(ps[:], ag1c[:, tsl],
                                     wo_sb[:, NC // 2 + hp, :],
                                     start=False, stop=True)
                    if hp == 0:
                        nc.vector.tensor_copy(out_acc[:, sb, :], ps[:])
                    else:
                        nc.vector.tensor_tensor(
                            out=out_acc[:, sb, :], in0=out_acc[:, sb, :],
                            in1=ps[:], op=add)
                    if final:
                        nc.sync.dma_start(out[bass.ts(sb, 128), :],
                                          out_acc[:, sb, :])

            pending = []
            for hp in range(NHP):
                # --- load this head-pair's weight slices ------------------
                wq_sb = wp.tile([128, NC, 128], BF16, tag="wq")
                nc.sync.dma_start(wq_sb[:], wqT[:, :, hp, :])
                wk_sb = wp.tile([128, NC, 128], BF16, tag="wk")
                nc.sync.dma_start(wk_sb[:], wkT[:, :, hp, :])
                wv_sb = wp.tile([128, NC, 128], BF16, tag="wv")
                nc.sync.dma_start(wv_sb[:], wvT[:, :, hp, :])

                # --- q/k projections + RoPE -------------------------------
                qT2 = qkp.tile([128, S], BF16, tag="qT")
                kT2 = qkp.tile([128, S], BF16, tag="kT")
                for w_sb, dst in ((wq_sb, qT2), (wk_sb, kT2)):
                    for j in range(NSQ):
                        jsl = bass.ts(j, SQT)
                        ps = psA.tile([128, SQT], F32, tag="psA")
                        for c in range(NC):
                            nc.tensor.matmul(
                                ps[:], w_sb[:, c, :], xt_sb[:, c, jsl],
                                start=(c == 0), stop=(c == NC - 1))
                        # evacuate PSUM first (frees the bank after one
                        # pass), then rotate in bf16:
                        # dst = q*cos + shuffle16(q)*sin_signed
                        qsb = smallp.tile([128, SQT], BF16, tag="qsb")
                        nc.vector.tensor_copy(qsb[:], ps[:])
                        t1 = ropep.tile([128, SQT], BF16, tag="t1")
                        nc.vector.tensor_tensor(
                            out=t1[:], in0=qsb[:], in1=cos_sb[:, jsl], op=mult)
                        sh = ropep.tile([128, SQT], BF16, tag="sh")
                        nc.vector.stream_shuffle(sh[:], qsb[:], SWAP16)
                        t2 = ropep.tile([128, SQT], BF16, tag="t2")
                        nc.gpsimd.tensor_tensor(
                            out=t2[:], in0=sh[:], in1=sin_sb[:, jsl], op=mult)
                        nc.gpsimd.tensor_tensor(
                            out=dst[:, jsl], in0=t1[:], in1=t2[:], op=add)

                # --- v projection + transpose + ones column ---------------
                vaug = vp.tile([128, NSB, 130], BF16, tag="vaug")
                nc.gpsimd.memset(vaug[:, :, 64], 1.0)
                nc.gpsimd.memset(vaug[:, :, 129], 1.0)
                for j in range(NSQ):
                    jsl = bass.ts(j, SQT)
                    ps = psA.tile([128, SQT], F32, tag="psA")
                    for c in range(NC):
                        nc.tensor.matmul(
                            ps[:], wv_sb[:, c, :], xt_sb[:, c, jsl],
                            start=(c == 0), stop=(c == NC - 1))
                    vt_sb = smallp.tile([128, SQT], BF16, tag="vt")
                    nc.vector.tensor_copy(vt_sb[:], ps[:])
                    for t in range(SQT // 128):
                        sb = j * (SQT // 128) + t
                        tp = psA.tile([128, 128], BF16, tag="psA")
                        nc.tensor.transpose(
                            tp[:], vt_sb[:, bass.ts(t, 128)], ident[:])
                        nc.vector.tensor_copy(vaug[:, sb, 0:64], tp[:, 0:64])
                        nc.vector.tensor_copy(vaug[:, sb, 65:129], tp[:, 64:128])

                # --- causal attention, scores held transposed -------------
                attnT2 = attnp.tile([128, S], BF16, tag="attnT")
                unnorm = unnp.tile([128, S], F32, tag="unnorm")
                for j in range(NSQ):
                    jsl = bass.ts(j, SQT)
                    pv0 = psPV.tile([128, SQT], F32, tag="pv")
                    pv1 = psPV.tile([128, SQT], F32, tag="pv")
                    n_sk = (SQT // 128) * j + 4
                    for i in range(n_sk):
                        qk2 = psQK.tile([128, 2 * SQT], F32, tag="qk")
                        for h in range(2):
                            nc.tensor.matmul(
                                qk2[:, bass.ts(h, SQT)],
                                kT2[64 * h:64 * h + 64, bass.ts(i, 128)],
                                qT2[64 * h:64 * h + 64, jsl],
                                start=True, stop=True)
                        pr2 = probsp.tile([128, 2 * SQT], BF16, tag="probs")
                        nc.scalar.activation(pr2[:], qk2[:], Exp, scale=0.125)
                        m = i - (SQT // 128) * j
                        if m >= 0:
                            nc.vector.tensor_tensor(
                                out=pr2[:], in0=pr2[:],
                                in1=mask_sb[:, m, :], op=mult)
                        for h, pv in ((0, pv0), (1, pv1)):
                            nc.tensor.matmul(
                                pv[0:65, :], vaug[:, i, 65 * h:65 * h + 65],
                                pr2[:, bass.ts(h, SQT)],
                                start=(i == 0), stop=(i == n_sk - 1))

                    # stage PV results + denominators (rows 0/32), free PSUM
                    den = normp.tile([64, SQT], F32, tag="den")
                    nc.gpsimd.memset(den[:], 1.0)
                    for h, pv in ((0, pv0), (1, pv1)):
                        nc.vector.tensor_copy(
                            unnorm[64 * h:64 * h + 64, jsl], pv[0:64, :])
                        nc.vector.tensor_copy(
                            den[32 * h:32 * h + 1, :], pv[64:65, :])

                    # Wo filler for already-gathered earlier slices
                    n_pop = 2 if hp == NHP - 1 and j > 0 else 1
                    for _ in range(n_pop):
                        if pending:
                            emit_wo_chunk(*pending.pop(0))

                    # normalize this slice and ship it
                    rec = normp.tile([64, SQT], F32, tag="recb")
                    nc.vector.reciprocal(rec[:], den[:])
                    for h in range(2):
                        if h == 0:
                            rin = rec[0:1, :]
                        else:
                            r1 = normp.tile([1, SQT], F32, tag="r1")
                            nc.vector.tensor_copy(r1[:], rec[32:33, :])
                            rin = r1[:]
                        rec128 = normp.tile([128, SQT], F32, tag="rec128")
                        nc.gpsimd.partition_broadcast(rec128[:], rin)
                        nc.vector.tensor_tensor(
                            out=attnT2[64 * h:64 * h + 64, jsl],
                            in0=unnorm[64 * h:64 * h + 64, jsl],
                            in1=rec128[64 * h:64 * h + 64, :], op=mult)

                    ag_in = dramp.tile([128, SQT], BF16, tag="ag_in")
                    nc.sync.dma_start(ag_in[:], attnT2[:, jsl])
                    ag_out = dramp.tile([2, 128, SQT], BF16, tag="ag_out")
                    nc.gpsimd.collective_compute(
                        "AllGather", mybir.AluOpType.bypass,
                        ins=[ag_in[:].opt()], outs=[ag_out[:].opt()],
                        replica_groups=groups)
                    ag0c = agp.tile([128, SQT], BF16, tag="ag0c")
                    nc.sync.dma_start(ag0c[:], ag_out[0])
                    ag1c = agp.tile([128, SQT], BF16, tag="ag1c")
                    nc.sync.dma_start(ag1c[:], ag_out[1])
                    pending.append((hp, j, ag0c, ag1c))

            while pending:
                emit_wo_chunk(*pending.pop(0))

    nc.compile()
    return nc


# ---------------------------------------------------------------------------
# Host-side sharding / unsharding
# ---------------------------------------------------------------------------

def _host_inputs(x, Wq, Wk, Wv, Wo, token_positions, n_cores, S):
    import ml_dtypes
    bf16 = ml_dtypes.bfloat16
    D = D_MODEL
    NC = D // 128
    NHP = 4

    # rope tables.  Partition layout within each head (64 partitions):
    # [e0..e15, o0..o15, e16..e31, o16..o31] -- the rotation partner sits
    # 16 partitions away inside the same 32-group, so the kernel's
    # stream_shuffle (a per-32-group lane shuffle) can realize the swap.
    pos = np.asarray(token_positions).astype(np.float32)  # (S,)
    i32 = np.arange(32, dtype=np.float32)
    inv_freq = ROPE_THETA ** (-i32 / 32.0)
    ang = pos[None, :] * inv_freq[:, None]              # (32, S)
    p = np.arange(128)
    pp = p % 64
    g, o = pp // 32, pp % 32
    freq_idx = 16 * g + (o % 16)                        # (128,)
    sign = np.where(o % 32 < 16, -1.0, 1.0)             # even slots: -sin
    cosT = np.cos(ang[freq_idx, :]).astype(bf16)        # (128, S)
    sinT = (sign[:, None] * np.sin(ang[freq_idx, :])).astype(bf16)

    # causal mask patterns for the 4 diagonal block offsets
    pcol = np.arange(128)[:, None]
    f = np.arange(SQT)[None, :]
    masks = np.stack([(pcol + 128 * m <= f) for m in range(4)], axis=1)
    masks = np.tile(masks, (1, 1, 2)).astype(bf16)      # (128, 4, 1024)

    # de-interleaving row permutation for q/k (see rope table comment)
    def qk_rows(grp):
        rows = []
        for h in range(8 * grp, 8 * grp + 8):
            rows += [h * DH + 2 * i for i in range(16)]
            rows += [h * DH + 2 * i + 1 for i in range(16)]
            rows += [h * DH + 2 * i for i in range(16, 32)]
            rows += [h * DH + 2 * i + 1 for i in range(16, 32)]
        return rows

    def wqk_layout(W, grp):
        # (D, 512) -> [128, NC, NHP, 128]
        t = W[qk_rows(grp), :].T.astype(bf16)
        return np.ascontiguousarray(
            t.reshape(NC, 128, NHP, 128).transpose(1, 0, 2, 3))

    def wv_layout(W, grp):
        t = W[512 * grp:512 * grp + 512, :].T.astype(bf16)
        return np.ascontiguousarray(
            t.reshape(NC, 128, NHP, 128).transpose(1, 0, 2, 3))

    def wo_layout(W, grp):
        t = W.T[:, 512 * grp:512 * grp + 512].astype(bf16)  # (D, 512)
        return np.ascontiguousarray(t.reshape(NC, 128, SQT).transpose(1, 0, 2))

    in_maps = []
    for c in range(n_cores):
        b, grp = c // 2, c % 2
        xb = np.ascontiguousarray(x[b].T).astype(bf16)  # (D, S)
        in_maps.append({
            "xT": np.ascontiguousarray(
                xb.reshape(NC, 128, S).transpose(1, 0, 2)),
            "wqT": wqk_layout(Wq, grp),
            "wkT": wqk_layout(Wk, grp),
            "wvT": wv_layout(Wv, grp),
            "woT": wo_layout(Wo, grp),
            "cosT": cosT,
            "sinT": sinT,
            "masks": masks,
        })
    return in_maps


def _assemble(results, n_cores, S):
    B = n_cores // 2
    full = np.empty((B, S, D_MODEL), dtype=np.float32)
    for c in range(n_cores):
        b, grp = c // 2, c % 2
        full[b, :, 512 * grp:512 * grp + 512] = results[c]["out"]
    return full


# ---------------------------------------------------------------------------
# Entry point
# ---------------------------------------------------------------------------

_NC_CACHE = {}


def _get_nc(n_cores, S):
    key = (n_cores, S)
    if key not in _NC_CACHE:
        _NC_CACHE[key] = build_kernel(n_cores, S)
    return _NC_CACHE[key]


def kernel(x, Wq, Wk, Wv, Wo, token_positions, _trace=False, _tmpdir=None):
    from concourse.bass_utils import run_bass_kernel_spmd

    x = np.asarray(x)
    B, S, D = x.shape
    n_cores = 2 * B
    nc = _get_nc(n_cores, S)
    in_maps = _host_inputs(np.asarray(x), np.asarray(Wq), np.asarray(Wk),
                           np.asarray(Wv), np.asarray(Wo),
                           np.asarray(token_positions), n_cores, S)
    res = run_bass_kernel_spmd(nc, in_maps, core_ids=list(range(n_cores)),
                               trace=_trace, tmpdir=_tmpdir)
    out = _assemble(res.results, n_cores, S)
    if _trace:
        return out, res
    return out



# revision 9
# speedup vs baseline: 1.0201x; 1.0201x over previous
"""Causal multi-head self-attention (B=4, S=2048, D=1024, H=16, RoPE) on 8
Trainium2 NeuronCores.

Sharding (hardcoded): core c handles batch b = c//2 and head group g = c%2
(8 of the 16 heads).  Data parallel over B, tensor parallel over heads for
the Wq/Wk/Wv projections and over Wo rows/columns: each core computes the
attention output for its 8 heads, the two cores of a pair AllGather their
(normalized) per-head-pair attention chunks once per 512-wide sq tile, and
each core then computes a disjoint 512-wide column slice of the final Wo
projection for its batch, so the host only concatenates slices.

Kernel structure (v2, tensor-engine-bound design):
  * sq-tile (j) OUTER loop, head-pair (hp) inner: per j, all four head
    pairs' q/k/v are projected (+RoPE), attention runs flash-style over
    transposed [sk 128, sq 512] score blocks, and the Wo partial for the
    whole j tile accumulates across all 8 dh-chunks in a single PSUM bank
    (no SBUF accumulator round-trips).
  * The QK -> exp -> PV chain is software-pipelined: QK(i+2) is emitted
    before PV(i) and projection/Wo matmuls for neighboring j tiles are
    interleaved as tensor-engine filler, so the PE never waits on the
    scalar-engine exp.
  * Diagonal score blocks are column-narrowed: block m only computes the
    valid [128m, 512) columns (shifted layout), exp covers the narrowed
    range, and only one 128-wide triangular strip per head needs a mask
    multiply.
  * Normalization: denominators ride as a 65th lhsT column through the PV
    matmul (PSUM row 64); reciprocal via the fast approx DVE op, broadcast
    across partitions on gpsimd, and a single fused multiply straight out
    of PV PSUM into the bf16 attnT tile.
"""

import numpy as np

D_MODEL = 1024
NUM_HEADS = 16
ROPE_THETA = 10000.0
DH = D_MODEL // NUM_HEADS  # 64
SQT = 512  # sq tile width (= PSUM bank width in f32)


# ---------------------------------------------------------------------------
# Device kernel builder
# ---------------------------------------------------------------------------

def build_kernel(n_cores: int = 8, S: int = 2048):
    import concourse.bass as bass
    import concourse.mybir as mybir
    import concourse.tile as tile
    from concourse import bacc
    from concourse.masks import make_identity

    F32 = mybir.dt.float32
    BF16 = mybir.dt.bfloat16
    Exp = mybir.ActivationFunctionType.Exp
    mult = mybir.AluOpType.mult
    add = mybir.AluOpType.add

    D = D_MODEL
    NC = D // 128          # 8 d-chunks
    NSB = S // 128         # s 128-blocks
    NSQ = S // SQT         # sq 512-tiles
    NHP = 4                # head pairs per core
    SWAP16 = list(range(16, 32)) + list(range(16))

    nc = bacc.Bacc("TRN2", target_bir_lowering=False, debug=False,
                   num_devices=n_cores)

    xT = nc.dram_tensor("xT", [128, NC, S], BF16, kind="ExternalInput")
    wqT = nc.dram_tensor("wqT", [128, NC, NHP, 128], BF16, kind="ExternalInput")
    wkT = nc.dram_tensor("wkT", [128, NC, NHP, 128], BF16, kind="ExternalInput")
    wvT = nc.dram_tensor("wvT", [128, NC, NHP, 128], BF16, kind="ExternalInput")
    woT = nc.dram_tensor("woT", [128, NC, SQT], BF16, kind="ExternalInput")
    cosT = nc.dram_tensor("cosT", [128, S], BF16, kind="ExternalInput")
    sinT = nc.dram_tensor("sinT", [128, S], BF16, kind="ExternalInput")
    triT = nc.dram_tensor("triT", [128, 128], BF16, kind="ExternalInput")
    out = nc.dram_tensor("out", [S, SQT], F32, kind="ExternalOutput")

    groups = [[2 * i, 2 * i + 1] for i in range(n_cores // 2)]

    with tile.TileContext(nc) as tc:
        with (
            tc.tile_pool(name="const", bufs=1) as constp,
            tc.tile_pool(name="qt", bufs=2) as qtp,
            tc.tile_pool(name="vt", bufs=2) as vtp,
            tc.tile_pool(name="probs", bufs=4) as probsp,
            tc.tile_pool(name="rope", bufs=2) as ropep,
            tc.tile_pool(name="attn", bufs=2) as attnp,
            tc.tile_pool(name="ag", bufs=2) as agp,
            tc.tile_pool(name="norm", bufs=2) as normp,
            tc.tile_pool(name="ost", bufs=2) as ostp,
            tc.tile_pool(name="psQK", bufs=2, space="PSUM") as psQK,
            tc.tile_pool(name="psPV", bufs=1, space="PSUM") as psPV,
            tc.tile_pool(name="psM", bufs=2, space="PSUM") as psM,
            tc.tile_pool(name="dram", bufs=2, space="DRAM") as dramp,
        ):
            # --- one-time loads -------------------------------------------
            xt_sb = constp.tile([128, NC, S], BF16, tag="xt")
            for c in range(NC):
                nc.sync.dma_start(xt_sb[:, c, :], xT[:, c, :])
            wq_sb = constp.tile([128, NC, NHP, 128], BF16, tag="wq")
            wk_sb = constp.tile([128, NC, NHP, 128], BF16, tag="wk")
            wv_sb = constp.tile([128, NC, NHP, 128], BF16, tag="wv")
            for hp in range(NHP):
                nc.sync.dma_start(wq_sb[:, :, hp, :], wqT[:, :, hp, :])
                nc.sync.dma_start(wk_sb[:, :, hp, :], wkT[:, :, hp, :])
                nc.sync.dma_start(wv_sb[:, :, hp, :], wvT[:, :, hp, :])
            cos_sb = constp.tile([128, S], BF16, tag="cos")
            nc.sync.dma_start(cos_sb[:], cosT[:])
            sin_sb = constp.tile([128, S], BF16, tag="sin")
            nc.sync.dma_start(sin_sb[:], sinT[:])
            tri_sb = constp.tile([128, 128], BF16, tag="tri")
            nc.sync.dma_start(tri_sb[:], triT[:])
            wo_sb = constp.tile([128, NC, SQT], BF16, tag="wo")
            nc.sync.dma_start(wo_sb[:], woT[:])
            ident = constp.tile([128, 128], BF16, tag="ident")
            make_identity(nc, ident[:])

            # persistent per-head-pair K / V
            kT = [constp.tile([128, S], BF16, tag=f"kT{hp}",
                              name=f"kT{hp}") for hp in range(NHP)]
            vaug = [constp.tile([128, NSB, 130], BF16, tag=f"vaug{hp}",
                                name=f"vaug{hp}") for hp in range(NHP)]
            for hp in range(NHP):
                nc.gpsimd.memset(vaug[hp][:, :, 64], 1.0)
                nc.gpsimd.memset(vaug[hp][:, :, 129], 1.0)

            qT_store = {}

            # --- projection emitters --------------------------------------
            def rope_into(dst_ap, ps, jsl):
                qsb = ropep.tile([128, SQT], BF16, tag="qsb")
                nc.vector.tensor_copy(qsb[:], ps[:])
                t1 = ropep.tile([128, SQT], BF16, tag="t1")
                nc.vector.tensor_tensor(out=t1[:], in0=qsb[:],
                                        in1=cos_sb[:, jsl], op=mult)
                sh = ropep.tile([128, SQT], BF16, tag="sh")
                nc.vector.stream_shuffle(sh[:], qsb[:], SWAP16)
                t2 = ropep.tile([128, SQT], BF16, tag="t2")
                nc.gpsimd.tensor_tensor(out=t2[:], in0=sh[:],
                                        in1=sin_sb[:, jsl], op=mult)
                nc.vector.tensor_tensor(out=dst_ap, in0=t1[:], in1=t2[:],
                                        op=add)

            def emit_q(hp, j):
                jsl = bass.ts(j, SQT)
                ps = psM.tile([128, SQT], F32, tag="psM")
                for c in range(NC):
                    nc.tensor.matmul(ps[:], wq_sb[:, c, hp, :],
                                     xt_sb[:, c, jsl],
                                     start=(c == 0), stop=(c == NC - 1))
                qt = qtp.tile([128, SQT], BF16, tag=f"qT{hp}")
                rope_into(qt[:], ps, jsl)
                qT_store[(hp, j)] = qt

            def emit_k(hp, j):
                jsl = bass.ts(j, SQT)
                ps = psM.tile([128, SQT], F32, tag="psM")
                for c in range(NC):
                    nc.tensor.matmul(ps[:], wk_sb[:, c, hp, :],
                                     xt_sb[:, c, jsl],
                                     start=(c == 0), stop=(c == NC - 1))
                rope_into(kT[hp][:, jsl], ps, jsl)

            def emit_v(hp, j):
                jsl = bass.ts(j, SQT)
                ps = psM.tile([128, SQT], F32, tag="psM")
                for c in range(NC):
                    nc.tensor.matmul(ps[:], wv_sb[:, c, hp, :],
                                     xt_sb[:, c, jsl],
                                     start=(c == 0), stop=(c == NC - 1))
                vt_sb = vtp.tile([128, SQT], BF16, tag="vt")
                nc.scalar.copy(vt_sb[:], ps[:])
                for t in range(SQT // 128):
                    sb = j * (SQT // 128) + t
                    tp = psM.tile([128, 128], BF16, tag="psM")
                    nc.tensor.transpose(
                        tp[:], vt_sb[:, bass.ts(t, 128)], ident[:])
                    nc.vector.tensor_copy(vaug[hp][:, sb, 0:64], tp[:, 0:64])
                    nc.vector.tensor_copy(vaug[hp][:, sb, 65:129],
                                          tp[:, 64:128])

            # --- Wo emitter (one unit per 128-row output block) ------------
            def make_wo_unit(sb, t, ag0c, ag1c):
                def unit():
                    tsl = bass.ts(t, 128)
                    ps = psM.tile([128, SQT], F32, tag="psM")
                    n8 = 0
                    for g, agc in ((0, ag0c), (1, ag1c)):
                        for hp in range(NHP):
                            nc.tensor.matmul(
                                ps[:], agc[:, hp, tsl],
                                wo_sb[:, NHP * g + hp, :],
                                start=(n8 == 0), stop=(n8 == 7))
                            n8 += 1
                    ost = ostp.tile([128, SQT], F32, tag="ost")
                    nc.scalar.copy(ost[:], ps[:])
                    nc.sync.dma_start(out[bass.ts(sb, 128), :], ost[:])
                return unit

            # --- filler machinery -----------------------------------------
            fillers = []
            state = {"blocks_left": 0, "err": 0.0}

            def pop_fillers(extra=0):
                for _ in range(extra):
                    if fillers:
                        fillers.pop(0)()
                if not fillers:
                    return
                state["err"] += len(fillers) / max(state["blocks_left"], 1)
                while state["err"] >= 1.0 and fillers:
                    state["err"] -= 1.0
                    fillers.pop(0)()

            # --- attention for one (hp, j) --------------------------------
            def attention(hp, j):
                n = 4 * j + 4
                qt = qT_store.pop((hp, j))
                pv = psPV.tile([128, 2 * SQT], F32, tag="pv")
                qk = {}
                probs = {}

                def emit_qk(i):
                    m = i - 4 * j
                    q2 = psQK.tile([128, 2 * SQT], F32, tag="qk")
                    if m <= 0:
                        for h in range(2):
                            nc.tensor.matmul(
                                q2[:, bass.ts(h, SQT)],
                                kT[hp][64 * h:64 * h + 64, bass.ts(i, 128)],
                                qt[64 * h:64 * h + 64, :],
                                start=True, stop=True)
                    else:
                        w = SQT - 128 * m
                        for h in range(2):
                            nc.tensor.matmul(
                                q2[:, SQT * h:SQT * h + w],
                                kT[hp][64 * h:64 * h + 64, bass.ts(i, 128)],
                                qt[64 * h:64 * h + 64, 128 * m:SQT],
                                start=True, stop=True)
                    qk[i] = q2

                def emit_exp(i):
                    m = i - 4 * j
                    pr = probsp.tile([128, 2 * SQT], BF16, tag="pr")
                    if m <= 0:
                        nc.scalar.activation(pr[:], qk[i][:], Exp, scale=0.125)
                    else:
                        w_tot = 2 * SQT - 128 * m
                        nc.scalar.activation(pr[:, 0:w_tot], qk[i][:, 0:w_tot],
                                             Exp, scale=0.125)
                    if m >= 0:
                        # one 128-wide triangular strip per head
                        nc.vector.tensor_tensor(
                            out=pr[:, 0:128], in0=pr[:, 0:128],
                            in1=tri_sb[:], op=mult)
                        nc.vector.tensor_tensor(
                            out=pr[:, SQT:SQT + 128], in0=pr[:, SQT:SQT + 128],
                            in1=tri_sb[:], op=mult)
                    del qk[i]
                    probs[i] = pr

                def emit_pv(i):
                    m = i - 4 * j
                    first = (i == 0)
                    last = (i == n - 1)
                    pr = probs.pop(i)
                    for h in range(2):
                        vsl = vaug[hp][:, i, 65 * h:65 * h + 65]
                        if m <= 0:
                            nc.tensor.matmul(
                                pv[0:65, SQT * h:SQT * h + SQT], vsl,
                                pr[:, bass.ts(h, SQT)],
                                start=first, stop=last)
                        else:
                            w = SQT - 128 * m
                            nc.tensor.matmul(
                                pv[0:65, SQT * h + 128 * m:SQT * h + SQT],
                                vsl, pr[:, SQT * h:SQT * h + w],
                                start=False, stop=last,
                                skip_group_check=True)

                emit_qk(0)
                pop_fillers(extra=1)
                emit_qk(1)
                pop_fillers(extra=1)
                for i in range(n):
                    emit_exp(i)
                    state["blocks_left"] -= 1
                    pop_fillers()
                    if i + 2 < n:
                        emit_qk(i + 2)
                    emit_pv(i)

                # --- normalize + emit into attnT tile ---------------------
                den = normp.tile([1, 2 * SQT], F32, tag="den")
                nc.vector.tensor_copy(den[:], pv[64:65, :])
                rec = normp.tile([1, 2 * SQT], F32, tag="rec")
                nc.vector.reciprocal_approx_fast(out=rec[:], in_=den[:])
                reca = normp.tile([64, SQT], F32, tag="reca")
                nc.gpsimd.partition_broadcast(reca[:], rec[0:1, 0:SQT],
                                              channels=64)
                recb = normp.tile([64, SQT], F32, tag="recb")
                nc.gpsimd.partition_broadcast(recb[:], rec[0:1, SQT:2 * SQT],
                                              channels=64)
                nc.vector.tensor_tensor(
                    out=attnT_j[0:64, hp, :], in0=pv[0:64, 0:SQT],
                    in1=reca[:], op=mult)
                nc.vector.tensor_tensor(
                    out=attnT_j[64:128, hp, :], in0=pv[0:64, SQT:2 * SQT],
                    in1=recb[:], op=mult)

            # --- prologue: projections for j = 0 --------------------------
            for hp in range(NHP):
                emit_q(hp, 0)
                emit_k(hp, 0)
                emit_v(hp, 0)

            # --- main loop ------------------------------------------------
            for j in range(NSQ):
                # queue next tile's projections as PE filler
                if j + 1 < NSQ:
                    for hp in range(NHP):
                        fillers.append(lambda hp=hp, j1=j + 1: emit_q(hp, j1))
                        fillers.append(lambda hp=hp, j1=j + 1: emit_k(hp, j1))
                        fillers.append(lambda hp=hp, j1=j + 1: emit_v(hp, j1))

                state["blocks_left"] = NHP * (4 * j + 4)
                attnT_j = attnp.tile([128, NHP, SQT], BF16, tag="attnT")
                ag0c = agp.tile([128, NHP, SQT], BF16, tag="ag0c")
                ag1c = agp.tile([128, NHP, SQT], BF16, tag="ag1c")
                for hp in range(NHP):
                    attention(hp, j)
                    # per-head-pair AllGather so the pair's Wo work can
                    # start before the whole tile finishes
                    ag_in = dramp.tile([128, SQT], BF16, tag="ag_in")
                    nc.sync.dma_start(ag_in[:], attnT_j[:, hp, :])
                    ag_out = dramp.tile([2, 128, SQT], BF16, tag="ag_out")
                    nc.gpsimd.collective_compute(
                        "AllGather", mybir.AluOpType.bypass,
                        ins=[ag_in[:].opt()], outs=[ag_out[:].opt()],
                        replica_groups=groups)
                    nc.sync.dma_start(ag0c[:, hp, :], ag_out[0])
                    nc.sync.dma_start(ag1c[:, hp, :], ag_out[1])

                # queue Wo for this tile (popped during the next tile)
                for t in range(SQT // 128):
                    sb = (SQT // 128) * j + t
                    fillers.append(make_wo_unit(sb, t, ag0c, ag1c))

            while fillers:
                fillers.pop(0)()

    nc.compile()
    return nc


# ---------------------------------------------------------------------------
# Host-side sharding / unsharding
# ---------------------------------------------------------------------------

def _host_inputs(x, Wq, Wk, Wv, Wo, token_positions, n_cores, S):
    import ml_dtypes
    bf16 = ml_dtypes.bfloat16
    D = D_MODEL
    NC = D // 128
    NHP = 4

    # rope tables.  Partition layout within each head (64 partitions):
    # [e0..e15, o0..o15, e16..e31, o16..o31] -- the rotation partner sits
    # 16 partitions away inside the same 32-group, so the kernel's
    # stream_shuffle (a per-32-group lane shuffle) can realize the swap.
    pos = np.asarray(token_positions).astype(np.float32)  # (S,)
    i32 = np.arange(32, dtype=np.float32)
    inv_freq = ROPE_THETA ** (-i32 / 32.0)
    ang = pos[None, :] * inv_freq[:, None]              # (32, S)
    p = np.arange(128)
    pp = p % 64
    g, o = pp // 32, pp % 32
    freq_idx = 16 * g + (o % 16)                        # (128,)
    sign = np.where(o % 32 < 16, -1.0, 1.0)             # even slots: -sin
    cosT = np.cos(ang[freq_idx, :]).astype(bf16)        # (128, S)
    sinT = (sign[:, None] * np.sin(ang[freq_idx, :])).astype(bf16)

    # triangular mask strip: valid (1.0) iff partition <= free col
    tri = (np.arange(128)[:, None] <= np.arange(128)[None, :]).astype(bf16)

    # de-interleaving row permutation for q/k (see rope table comment)
    def qk_rows(grp):
        rows = []
        for h in range(8 * grp, 8 * grp + 8):
            rows += [h * DH + 2 * i for i in range(16)]
            rows += [h * DH + 2 * i + 1 for i in range(16)]
            rows += [h * DH + 2 * i for i in range(16, 32)]
            rows += [h * DH + 2 * i + 1 for i in range(16, 32)]
        return rows

    def wqk_layout(W, grp):
        # (D, 512) -> [128, NC, NHP, 128]
        t = W[qk_rows(grp), :].T.astype(bf16)
        return np.ascontiguousarray(
            t.reshape(NC, 128, NHP, 128).transpose(1, 0, 2, 3))

    def wv_layout(W, grp):
        t = W[512 * grp:512 * grp + 512, :].T.astype(bf16)
        return np.ascontiguousarray(
            t.reshape(NC, 128, NHP, 128).transpose(1, 0, 2, 3))

    def wo_layout(W, grp):
        t = W.T[:, 512 * grp:512 * grp + 512].astype(bf16)  # (D, 512)
        return np.ascontiguousarray(t.reshape(NC, 128, SQT).transpose(1, 0, 2))

    in_maps = []
    for c in range(n_cores):
        b, grp = c // 2, c % 2
        xb = np.ascontiguousarray(x[b].T).astype(bf16)  # (D, S)
        in_maps.append({
            "xT": np.ascontiguousarray(
                xb.reshape(NC, 128, S).transpose(1, 0, 2)),
            "wqT": wqk_layout(Wq, grp),
            "wkT": wqk_layout(Wk, grp),
            "wvT": wv_layout(Wv, grp),
            "woT": wo_layout(Wo, grp),
            "cosT": cosT,
            "sinT": sinT,
            "triT": tri,
        })
    return in_maps


def _assemble(results, n_cores, S):
    B = n_cores // 2
    full = np.empty((B, S, D_MODEL), dtype=np.float32)
    for c in range(n_cores):
        b, grp = c // 2, c % 2
        full[b, :, 512 * grp:512 * grp + 512] = results[c]["out"]
    return full


# ---------------------------------------------------------------------------
# Entry point
# ---------------------------------------------------------------------------

_NC_CACHE = {}


def _get_nc(n_cores, S):
    key = (n_cores, S)
    if key not in _NC_CACHE:
        _NC_CACHE[key] = build_kernel(n_cores, S)
    return _NC_CACHE[key]


def kernel(x, Wq, Wk, Wv, Wo, token_positions, _trace=False, _tmpdir=None):
    from concourse.bass_utils import run_bass_kernel_spmd

    x = np.asarray(x)
    B, S, D = x.shape
    n_cores = 2 * B
    nc = _get_nc(n_cores, S)
    in_maps = _host_inputs(np.asarray(x), np.asarray(Wq), np.asarray(Wk),
                           np.asarray(Wv), np.asarray(Wo),
                           np.asarray(token_positions), n_cores, S)
    res = run_bass_kernel_spmd(nc, in_maps, core_ids=list(range(n_cores)),
                               trace=_trace, tmpdir=_tmpdir)
    out = _assemble(res.results, n_cores, S)
    if _trace:
        return out, res
    return out


# revision 13
# speedup vs baseline: 1.0401x; 1.0196x over previous
"""Causal multi-head self-attention (B=4, S=2048, D=1024, H=16, RoPE) on 8
Trainium2 NeuronCores.

Sharding (hardcoded): core c handles batch b = c//2 and head group g = c%2
(8 of the 16 heads).  Data parallel over B, tensor parallel over heads for
the Wq/Wk/Wv projections and over Wo rows/columns: each core computes the
attention output for its 8 heads, the two cores of a pair AllGather their
(normalized) per-head-pair attention chunks once per 512-wide sq tile, and
each core then computes a disjoint 512-wide column slice of the final Wo
projection for its batch, so the host only concatenates slices.

Kernel structure (v2, tensor-engine-bound design):
  * sq-tile (j) OUTER loop, head-pair (hp) inner: per j, all four head
    pairs' q/k/v are projected (+RoPE), attention runs flash-style over
    transposed [sk 128, sq 512] score blocks, and the Wo partial for the
    whole j tile accumulates across all 8 dh-chunks in a single PSUM bank
    (no SBUF accumulator round-trips).
  * The QK -> exp -> PV chain is software-pipelined: QK(i+2) is emitted
    before PV(i) and projection/Wo matmuls for neighboring j tiles are
    interleaved as tensor-engine filler, so the PE never waits on the
    scalar-engine exp.
  * Diagonal score blocks are column-narrowed: block m only computes the
    valid [128m, 512) columns (shifted layout), exp covers the narrowed
    range, and only one 128-wide triangular strip per head needs a mask
    multiply.
  * Normalization: denominators ride as a 65th lhsT column through the PV
    matmul (PSUM row 64); reciprocal via the fast approx DVE op, broadcast
    across partitions on gpsimd, and a single fused multiply straight out
    of PV PSUM into the bf16 attnT tile.
"""

import numpy as np

D_MODEL = 1024
NUM_HEADS = 16
ROPE_THETA = 10000.0
DH = D_MODEL // NUM_HEADS  # 64
SQT = 512  # sq tile width (= PSUM bank width in f32)


# ---------------------------------------------------------------------------
# Device kernel builder
# ---------------------------------------------------------------------------

def build_kernel(n_cores: int = 8, S: int = 2048):
    import concourse.bass as bass
    import concourse.mybir as mybir
    import concourse.tile as tile
    from concourse import bacc
    from concourse.masks import make_identity

    F32 = mybir.dt.float32
    BF16 = mybir.dt.bfloat16
    Exp = mybir.ActivationFunctionType.Exp
    mult = mybir.AluOpType.mult
    add = mybir.AluOpType.add

    D = D_MODEL
    NC = D // 128          # 8 d-chunks
    NSB = S // 128         # s 128-blocks
    NSQ = S // SQT         # sq 512-tiles
    NHP = 4                # head pairs per core
    SWAP16 = list(range(16, 32)) + list(range(16))

    nc = bacc.Bacc("TRN2", target_bir_lowering=False, debug=False,
                   num_devices=n_cores)

    xT = nc.dram_tensor("xT", [128, NC, S], BF16, kind="ExternalInput")
    wqT = nc.dram_tensor("wqT", [128, NC, NHP, 128], BF16, kind="ExternalInput")
    wkT = nc.dram_tensor("wkT", [128, NC, NHP, 128], BF16, kind="ExternalInput")
    wvT = nc.dram_tensor("wvT", [128, NC, NHP, 128], BF16, kind="ExternalInput")
    woT = nc.dram_tensor("woT", [128, NC, SQT], BF16, kind="ExternalInput")
    cosT = nc.dram_tensor("cosT", [128, S], BF16, kind="ExternalInput")
    sinT = nc.dram_tensor("sinT", [128, S], BF16, kind="ExternalInput")
    triT = nc.dram_tensor("triT", [128, 128], BF16, kind="ExternalInput")
    out = nc.dram_tensor("out", [S, SQT], F32, kind="ExternalOutput")

    groups = [[2 * i, 2 * i + 1] for i in range(n_cores // 2)]

    with tile.TileContext(nc) as tc:
        with (
            tc.tile_pool(name="const", bufs=1) as constp,
            tc.tile_pool(name="qt", bufs=2) as qtp,
            tc.tile_pool(name="vt", bufs=2) as vtp,
            tc.tile_pool(name="probs", bufs=4) as probsp,
            tc.tile_pool(name="rope", bufs=2) as ropep,
            tc.tile_pool(name="attn", bufs=2) as attnp,
            tc.tile_pool(name="ag", bufs=2) as agp,
            tc.tile_pool(name="norm", bufs=2) as normp,
            tc.tile_pool(name="ost", bufs=2) as ostp,
            tc.tile_pool(name="psQK", bufs=2, space="PSUM") as psQK,
            tc.tile_pool(name="psPV", bufs=1, space="PSUM") as psPV,
            tc.tile_pool(name="psM", bufs=2, space="PSUM") as psM,
            tc.tile_pool(name="dram", bufs=2, space="DRAM") as dramp,
        ):
            # --- one-time loads -------------------------------------------
            xt_sb = constp.tile([128, NC, S], BF16, tag="xt")
            for c in range(NC):
                nc.sync.dma_start(xt_sb[:, c, :], xT[:, c, :])
            wq_sb = constp.tile([128, NC, NHP, 128], BF16, tag="wq")
            wk_sb = constp.tile([128, NC, NHP, 128], BF16, tag="wk")
            wv_sb = constp.tile([128, NC, NHP, 128], BF16, tag="wv")
            for hp in range(NHP):
                nc.sync.dma_start(wq_sb[:, :, hp, :], wqT[:, :, hp, :])
                nc.sync.dma_start(wk_sb[:, :, hp, :], wkT[:, :, hp, :])
                nc.sync.dma_start(wv_sb[:, :, hp, :], wvT[:, :, hp, :])
            cos_sb = constp.tile([128, S], BF16, tag="cos")
            nc.sync.dma_start(cos_sb[:], cosT[:])
            sin_sb = constp.tile([128, S], BF16, tag="sin")
            nc.sync.dma_start(sin_sb[:], sinT[:])
            tri_sb = constp.tile([128, 128], BF16, tag="tri")
            nc.sync.dma_start(tri_sb[:], triT[:])
            wo_sb = constp.tile([128, NC, SQT], BF16, tag="wo")
            nc.sync.dma_start(wo_sb[:], woT[:])
            ident = constp.tile([128, 128], BF16, tag="ident")
            make_identity(nc, ident[:])

            # persistent per-head-pair K / V
            kT = [constp.tile([128, S], BF16, tag=f"kT{hp}",
                              name=f"kT{hp}") for hp in range(NHP)]
            vaug = [constp.tile([128, NSB, 130], BF16, tag=f"vaug{hp}",
                                name=f"vaug{hp}") for hp in range(NHP)]
            for hp in range(NHP):
                nc.gpsimd.memset(vaug[hp][:, :, 64], 1.0)
                nc.gpsimd.memset(vaug[hp][:, :, 129], 1.0)

            qT_store = {}

            # --- projection emitters --------------------------------------
            def rope_into(dst_ap, ps, jsl):
                qsb = ropep.tile([128, SQT], BF16, tag="qsb")
                nc.vector.tensor_copy(qsb[:], ps[:])
                t1 = ropep.tile([128, SQT], BF16, tag="t1")
                nc.vector.tensor_tensor(out=t1[:], in0=qsb[:],
                                        in1=cos_sb[:, jsl], op=mult)
                sh = ropep.tile([128, SQT], BF16, tag="sh")
                nc.vector.stream_shuffle(sh[:], qsb[:], SWAP16)
                t2 = ropep.tile([128, SQT], BF16, tag="t2")
                nc.gpsimd.tensor_tensor(out=t2[:], in0=sh[:],
                                        in1=sin_sb[:, jsl], op=mult)
                nc.vector.tensor_tensor(out=dst_ap, in0=t1[:], in1=t2[:],
                                        op=add)

            def emit_q(hp, j):
                jsl = bass.ts(j, SQT)
                ps = psM.tile([128, SQT], F32, tag="psM")
                for c in range(NC):
                    nc.tensor.matmul(ps[:], wq_sb[:, c, hp, :],
                                     xt_sb[:, c, jsl],
                                     start=(c == 0), stop=(c == NC - 1))
                    yield
                qt = qtp.tile([128, SQT], BF16, tag=f"qT{hp}")
                rope_into(qt[:], ps, jsl)
                qT_store[(hp, j)] = qt

            def emit_k(hp, j):
                jsl = bass.ts(j, SQT)
                ps = psM.tile([128, SQT], F32, tag="psM")
                for c in range(NC):
                    nc.tensor.matmul(ps[:], wk_sb[:, c, hp, :],
                                     xt_sb[:, c, jsl],
                                     start=(c == 0), stop=(c == NC - 1))
                    yield
                rope_into(kT[hp][:, jsl], ps, jsl)

            def emit_v(hp, j):
                jsl = bass.ts(j, SQT)
                ps = psM.tile([128, SQT], F32, tag="psM")
                for c in range(NC):
                    nc.tensor.matmul(ps[:], wv_sb[:, c, hp, :],
                                     xt_sb[:, c, jsl],
                                     start=(c == 0), stop=(c == NC - 1))
                    yield
                vt_sb = vtp.tile([128, SQT], BF16, tag="vt")
                nc.scalar.copy(vt_sb[:], ps[:])
                for t in range(SQT // 128):
                    sb = j * (SQT // 128) + t
                    tp = psM.tile([128, 128], BF16, tag="psM")
                    nc.tensor.transpose(
                        tp[:], vt_sb[:, bass.ts(t, 128)], ident[:])
                    nc.vector.tensor_copy(vaug[hp][:, sb, 0:64], tp[:, 0:64])
                    nc.vector.tensor_copy(vaug[hp][:, sb, 65:129],
                                          tp[:, 64:128])
                    yield

            # --- Wo emitter (one unit per 128-row output block) ------------
            def emit_wo(sb, t, ag0c, ag1c):
                tsl = bass.ts(t, 128)
                ps = psM.tile([128, SQT], F32, tag="psM")
                n8 = 0
                for g, agc in ((0, ag0c), (1, ag1c)):
                    for hp in range(NHP):
                        nc.tensor.matmul(
                            ps[:], agc[:, hp, tsl],
                            wo_sb[:, NHP * g + hp, :],
                            start=(n8 == 0), stop=(n8 == 7))
                        n8 += 1
                        yield
                ost = ostp.tile([128, SQT], F32, tag="ost")
                nc.scalar.copy(ost[:], ps[:])
                nc.sync.dma_start(out[bass.ts(sb, 128), :], ost[:])

            # --- filler machinery: generators yielding per-matmul ---------
            # Interleaves projection/Wo matmuls one at a time between
            # attention blocks so the in-order PE queue always has ready
            # work while exp runs on the scalar engine.
            fillers = []          # list of [generator, approx_steps_left]
            state = {"blocks_left": 0, "err": 0.0}

            def _steps_left():
                return sum(s for _, s in fillers)

            def _advance(n):
                while n > 0 and fillers:
                    ent = fillers[0]
                    try:
                        next(ent[0])
                        ent[1] = max(ent[1] - 1, 0)
                        n -= 1
                    except StopIteration:
                        fillers.pop(0)

            def pop_fillers(extra=0):
                if extra:
                    _advance(extra)
                if not fillers:
                    return
                state["err"] += _steps_left() / max(state["blocks_left"], 1)
                k = int(state["err"])
                if k > 0:
                    state["err"] -= k
                    _advance(k)

            # --- attention for one (hp, j) --------------------------------
            def attention(hp, j):
                n = 4 * j + 4
                qt = qT_store.pop((hp, j))
                pv = psPV.tile([128, 2 * SQT], F32, tag="pv")
                qk = {}
                probs = {}

                def emit_qk(i):
                    m = i - 4 * j
                    q2 = psQK.tile([128, 2 * SQT], F32, tag="qk")
                    if m <= 0:
                        for h in range(2):
                            nc.tensor.matmul(
                                q2[:, bass.ts(h, SQT)],
                                kT[hp][64 * h:64 * h + 64, bass.ts(i, 128)],
                                qt[64 * h:64 * h + 64, :],
                                start=True, stop=True)
                    else:
                        w = SQT - 128 * m
                        for h in range(2):
                            nc.tensor.matmul(
                                q2[:, SQT * h:SQT * h + w],
                                kT[hp][64 * h:64 * h + 64, bass.ts(i, 128)],
                                qt[64 * h:64 * h + 64, 128 * m:SQT],
                                start=True, stop=True)
                    qk[i] = q2

                def emit_exp(i):
                    m = i - 4 * j
                    pr = probsp.tile([128, 2 * SQT], BF16, tag="pr")
                    if m <= 0:
                        nc.scalar.activation(pr[:], qk[i][:], Exp, scale=0.125)
                    else:
                        w_tot = 2 * SQT - 128 * m
                        nc.scalar.activation(pr[:, 0:w_tot], qk[i][:, 0:w_tot],
                                             Exp, scale=0.125)
                    if m >= 0:
                        # one 128-wide triangular strip per head
                        nc.vector.tensor_tensor(
                            out=pr[:, 0:128], in0=pr[:, 0:128],
                            in1=tri_sb[:], op=mult)
                        nc.vector.tensor_tensor(
                            out=pr[:, SQT:SQT + 128], in0=pr[:, SQT:SQT + 128],
                            in1=tri_sb[:], op=mult)
                    del qk[i]
                    probs[i] = pr

                def emit_pv(i):
                    m = i - 4 * j
                    first = (i == 0)
                    last = (i == n - 1)
                    pr = probs.pop(i)
                    for h in range(2):
                        vsl = vaug[hp][:, i, 65 * h:65 * h + 65]
                        if m <= 0:
                            nc.tensor.matmul(
                                pv[0:65, SQT * h:SQT * h + SQT], vsl,
                                pr[:, bass.ts(h, SQT)],
                                start=first, stop=last)
                        else:
                            w = SQT - 128 * m
                            nc.tensor.matmul(
                                pv[0:65, SQT * h + 128 * m:SQT * h + SQT],
                                vsl, pr[:, SQT * h:SQT * h + w],
                                start=False, stop=last,
                                skip_group_check=True)

                emit_qk(0)
                pop_fillers(extra=1)
                emit_qk(1)
                pop_fillers(extra=1)
                for i in range(n):
                    emit_exp(i)
                    state["blocks_left"] -= 1
                    pop_fillers()
                    if i + 2 < n:
                        emit_qk(i + 2)
                    emit_pv(i)

                # --- normalize + emit into attnT tile ---------------------
                den = normp.tile([1, 2 * SQT], F32, tag="den")
                nc.vector.tensor_copy(den[:], pv[64:65, :])
                rec = normp.tile([1, 2 * SQT], F32, tag="rec")
                nc.vector.reciprocal_approx_fast(out=rec[:], in_=den[:])
                reca = normp.tile([64, SQT], F32, tag="reca")
                nc.gpsimd.partition_broadcast(reca[:], rec[0:1, 0:SQT],
                                              channels=64)
                recb = normp.tile([64, SQT], F32, tag="recb")
                nc.gpsimd.partition_broadcast(recb[:], rec[0:1, SQT:2 * SQT],
                                              channels=64)
                nc.vector.tensor_tensor(
                    out=attnT_j[0:64, hp, :], in0=pv[0:64, 0:SQT],
                    in1=reca[:], op=mult)
                nc.vector.tensor_tensor(
                    out=attnT_j[64:128, hp, :], in0=pv[0:64, SQT:2 * SQT],
                    in1=recb[:], op=mult)

            def run_all(gen):
                for _ in gen:
                    pass

            # --- prologue: projections for j = 0 --------------------------
            for hp in range(NHP):
                run_all(emit_q(hp, 0))
                run_all(emit_k(hp, 0))
                run_all(emit_v(hp, 0))

            # --- main loop ------------------------------------------------
            for j in range(NSQ):
                # queue next tile's projections as PE filler
                if j + 1 < NSQ:
                    for hp in range(NHP):
                        fillers.append([emit_q(hp, j + 1), 9])
                        fillers.append([emit_k(hp, j + 1), 9])
                        fillers.append([emit_v(hp, j + 1), 13])

                state["blocks_left"] = NHP * (4 * j + 4)
                attnT_j = attnp.tile([128, NHP, SQT], BF16, tag="attnT")
                ag0c = agp.tile([128, NHP, SQT], BF16, tag="ag0c")
                ag1c = agp.tile([128, NHP, SQT], BF16, tag="ag1c")
                for hp in range(NHP):
                    attention(hp, j)
                    # per-head-pair AllGather so the pair's Wo work can
                    # start before the whole tile finishes
                    ag_in = dramp.tile([128, SQT], BF16, tag="ag_in")
                    nc.sync.dma_start(ag_in[:], attnT_j[:, hp, :])
                    ag_out = dramp.tile([2, 128, SQT], BF16, tag="ag_out")
                    nc.gpsimd.collective_compute(
                        "AllGather", mybir.AluOpType.bypass,
                        ins=[ag_in[:].opt()], outs=[ag_out[:].opt()],
                        replica_groups=groups)
                    nc.sync.dma_start(ag0c[:, hp, :], ag_out[0])
                    nc.sync.dma_start(ag1c[:, hp, :], ag_out[1])

                # queue Wo for this tile (popped during the next tile)
                for t in range(SQT // 128):
                    sb = (SQT // 128) * j + t
                    fillers.append([emit_wo(sb, t, ag0c, ag1c), 9])

            while fillers:
                _advance(1)

    nc.compile()
    return nc


# ---------------------------------------------------------------------------
# Host-side sharding / unsharding
# ---------------------------------------------------------------------------

def _host_inputs(x, Wq, Wk, Wv, Wo, token_positions, n_cores, S):
    import ml_dtypes
    bf16 = ml_dtypes.bfloat16
    D = D_MODEL
    NC = D // 128
    NHP = 4

    # rope tables.  Partition layout within each head (64 partitions):
    # [e0..e15, o0..o15, e16..e31, o16..o31] -- the rotation partner sits
    # 16 partitions away inside the same 32-group, so the kernel's
    # stream_shuffle (a per-32-group lane shuffle) can realize the swap.
    pos = np.asarray(token_positions).astype(np.float32)  # (S,)
    i32 = np.arange(32, dtype=np.float32)
    inv_freq = ROPE_THETA ** (-i32 / 32.0)
    ang = pos[None, :] * inv_freq[:, None]              # (32, S)
    p = np.arange(128)
    pp = p % 64
    g, o = pp // 32, pp % 32
    freq_idx = 16 * g + (o % 16)                        # (128,)
    sign = np.where(o % 32 < 16, -1.0, 1.0)             # even slots: -sin
    cosT = np.cos(ang[freq_idx, :]).astype(bf16)        # (128, S)
    sinT = (sign[:, None] * np.sin(ang[freq_idx, :])).astype(bf16)

    # triangular mask strip: valid (1.0) iff partition <= free col
    tri = (np.arange(128)[:, None] <= np.arange(128)[None, :]).astype(bf16)

    # de-interleaving row permutation for q/k (see rope table comment)
    def qk_rows(grp):
        rows = []
        for h in range(8 * grp, 8 * grp + 8):
            rows += [h * DH + 2 * i for i in range(16)]
            rows += [h * DH + 2 * i + 1 for i in range(16)]
            rows += [h * DH + 2 * i for i in range(16, 32)]
            rows += [h * DH + 2 * i + 1 for i in range(16, 32)]
        return rows

    def wqk_layout(W, grp):
        # (D, 512) -> [128, NC, NHP, 128]
        t = W[qk_rows(grp), :].T.astype(bf16)
        return np.ascontiguousarray(
            t.reshape(NC, 128, NHP, 128).transpose(1, 0, 2, 3))

    def wv_layout(W, grp):
        t = W[512 * grp:512 * grp + 512, :].T.astype(bf16)
        return np.ascontiguousarray(
            t.reshape(NC, 128, NHP, 128).transpose(1, 0, 2, 3))

    def wo_layout(W, grp):
        t = W.T[:, 512 * grp:512 * grp + 512].astype(bf16)  # (D, 512)
        return np.ascontiguousarray(t.reshape(NC, 128, SQT).transpose(1, 0, 2))

    in_maps = []
    for c in range(n_cores):
        b, grp = c // 2, c % 2
        xb = np.ascontiguousarray(x[b].T).astype(bf16)  # (D, S)
        in_maps.append({
            "xT": np.ascontiguousarray(
                xb.reshape(NC, 128, S).transpose(1, 0, 2)),
            "wqT": wqk_layout(Wq, grp),
            "wkT": wqk_layout(Wk, grp),
            "wvT": wv_layout(Wv, grp),
            "woT": wo_layout(Wo, grp),
            "cosT": cosT,
            "sinT": sinT,
            "triT": tri,
        })
    return in_maps


def _assemble(results, n_cores, S):
    B = n_cores // 2
    full = np.empty((B, S, D_MODEL), dtype=np.float32)
    for c in range(n_cores):
        b, grp = c // 2, c % 2
        full[b, :, 512 * grp:512 * grp + 512] = results[c]["out"]
    return full


# ---------------------------------------------------------------------------
# Entry point
# ---------------------------------------------------------------------------

_NC_CACHE = {}


def _get_nc(n_cores, S):
    key = (n_cores, S)
    if key not in _NC_CACHE:
        _NC_CACHE[key] = build_kernel(n_cores, S)
    return _NC_CACHE[key]


def kernel(x, Wq, Wk, Wv, Wo, token_positions, _trace=False, _tmpdir=None):
    from concourse.bass_utils import run_bass_kernel_spmd

    x = np.asarray(x)
    B, S, D = x.shape
    n_cores = 2 * B
    nc = _get_nc(n_cores, S)
    in_maps = _host_inputs(np.asarray(x), np.asarray(Wq), np.asarray(Wk),
                           np.asarray(Wv), np.asarray(Wo),
                           np.asarray(token_positions), n_cores, S)
    res = run_bass_kernel_spmd(nc, in_maps, core_ids=list(range(n_cores)),
                               trace=_trace, tmpdir=_tmpdir)
    out = _assemble(res.results, n_cores, S)
    if _trace:
        return out, res
    return out


# revision 19
# speedup vs baseline: 1.0484x; 1.0080x over previous
"""Causal multi-head self-attention (B=4, S=2048, D=1024, H=16, RoPE) on 8
Trainium2 NeuronCores.

Sharding (hardcoded): core c handles batch b = c//2 and head group g = c%2
(8 of the 16 heads).  Data parallel over B, tensor parallel over heads for
the Wq/Wk/Wv projections and over Wo rows/columns: each core computes the
attention output for its 8 heads, the two cores of a pair AllGather their
(normalized) per-head-pair attention chunks once per 512-wide sq tile, and
each core then computes a disjoint 512-wide column slice of the final Wo
projection for its batch, so the host only concatenates slices.

Kernel structure (v2, tensor-engine-bound design):
  * sq-tile (j) OUTER loop, head-pair (hp) inner: per j, all four head
    pairs' q/k/v are projected (+RoPE), attention runs flash-style over
    transposed [sk 128, sq 512] score blocks, and the Wo partial for the
    whole j tile accumulates across all 8 dh-chunks in a single PSUM bank
    (no SBUF accumulator round-trips).
  * The QK -> exp -> PV chain is software-pipelined: QK(i+2) is emitted
    before PV(i) and projection/Wo matmuls for neighboring j tiles are
    interleaved as tensor-engine filler, so the PE never waits on the
    scalar-engine exp.
  * Diagonal score blocks are column-narrowed: block m only computes the
    valid [128m, 512) columns (shifted layout), exp covers the narrowed
    range, and only one 128-wide triangular strip per head needs a mask
    multiply.
  * Normalization: denominators ride as a 65th lhsT column through the PV
    matmul (PSUM row 64); reciprocal via the fast approx DVE op, broadcast
    across partitions on gpsimd, and a single fused multiply straight out
    of PV PSUM into the bf16 attnT tile.
"""

import numpy as np

D_MODEL = 1024
NUM_HEADS = 16
ROPE_THETA = 10000.0
DH = D_MODEL // NUM_HEADS  # 64
SQT = 512  # sq tile width (= PSUM bank width in f32)


# ---------------------------------------------------------------------------
# Device kernel builder
# ---------------------------------------------------------------------------

def build_kernel(n_cores: int = 8, S: int = 2048):
    import concourse.bass as bass
    import concourse.mybir as mybir
    import concourse.tile as tile
    from concourse import bacc
    from concourse.masks import make_identity

    F32 = mybir.dt.float32
    BF16 = mybir.dt.bfloat16
    Exp = mybir.ActivationFunctionType.Exp
    mult = mybir.AluOpType.mult
    add = mybir.AluOpType.add

    D = D_MODEL
    NC = D // 128          # 8 d-chunks
    NSB = S // 128         # s 128-blocks
    NSQ = S // SQT         # sq 512-tiles
    NHP = 4                # head pairs per core
    SWAP16 = list(range(16, 32)) + list(range(16))

    nc = bacc.Bacc("TRN2", target_bir_lowering=False, debug=False,
                   num_devices=n_cores)

    xT = nc.dram_tensor("xT", [128, NC, S], BF16, kind="ExternalInput")
    wqT = nc.dram_tensor("wqT", [128, NC, NHP, 128], BF16, kind="ExternalInput")
    wkT = nc.dram_tensor("wkT", [128, NC, NHP, 128], BF16, kind="ExternalInput")
    wvT = nc.dram_tensor("wvT", [128, NC, NHP, 128], BF16, kind="ExternalInput")
    woT = nc.dram_tensor("woT", [128, NC, SQT], BF16, kind="ExternalInput")
    cosT = nc.dram_tensor("cosT", [128, S], BF16, kind="ExternalInput")
    sinT = nc.dram_tensor("sinT", [128, S], BF16, kind="ExternalInput")
    # triangle-bias matmul constants: biasL.T @ biasR = -B on the
    # strictly-lower-triangular (masked) region of a 128x128 block
    biasLT = nc.dram_tensor("biasLT", [128, 128], BF16, kind="ExternalInput")
    biasRT = nc.dram_tensor("biasRT", [128, 128], BF16, kind="ExternalInput")
    out = nc.dram_tensor("out", [S, SQT], F32, kind="ExternalOutput")

    groups = [[2 * i, 2 * i + 1] for i in range(n_cores // 2)]

    with tile.TileContext(nc) as tc:
        with (
            tc.tile_pool(name="const", bufs=1) as constp,
            tc.tile_pool(name="qt", bufs=2) as qtp,
            tc.tile_pool(name="vt", bufs=2) as vtp,
            tc.tile_pool(name="probs", bufs=4) as probsp,
            tc.tile_pool(name="rope", bufs=2) as ropep,
            tc.tile_pool(name="attn", bufs=2) as attnp,
            tc.tile_pool(name="ag", bufs=2) as agp,
            tc.tile_pool(name="norm", bufs=2) as normp,
            tc.tile_pool(name="ost", bufs=2) as ostp,
            tc.tile_pool(name="psQK", bufs=2, space="PSUM") as psQK,
            tc.tile_pool(name="psPV", bufs=1, space="PSUM") as psPV,
            tc.tile_pool(name="psM", bufs=2, space="PSUM") as psM,
            tc.tile_pool(name="dram", bufs=2, space="DRAM") as dramp,
        ):
            # --- one-time loads -------------------------------------------
            xt_sb = constp.tile([128, NC, S], BF16, tag="xt")
            for c in range(NC):
                nc.sync.dma_start(xt_sb[:, c, :], xT[:, c, :])
            wq_sb = constp.tile([128, NC, NHP, 128], BF16, tag="wq")
            wk_sb = constp.tile([128, NC, NHP, 128], BF16, tag="wk")
            wv_sb = constp.tile([128, NC, NHP, 128], BF16, tag="wv")
            for hp in range(NHP):
                nc.sync.dma_start(wq_sb[:, :, hp, :], wqT[:, :, hp, :])
                nc.sync.dma_start(wk_sb[:, :, hp, :], wkT[:, :, hp, :])
                nc.sync.dma_start(wv_sb[:, :, hp, :], wvT[:, :, hp, :])
            cos_sb = constp.tile([128, S], BF16, tag="cos")
            nc.sync.dma_start(cos_sb[:], cosT[:])
            sin_sb = constp.tile([128, S], BF16, tag="sin")
            nc.sync.dma_start(sin_sb[:], sinT[:])
            biasL = constp.tile([128, 128], BF16, tag="biasL")
            nc.sync.dma_start(biasL[:], biasLT[:])
            biasR = constp.tile([128, 128], BF16, tag="biasR")
            nc.sync.dma_start(biasR[:], biasRT[:])
            wo_sb = constp.tile([128, NC, SQT], BF16, tag="wo")
            nc.sync.dma_start(wo_sb[:], woT[:])
            ident = constp.tile([128, 128], BF16, tag="ident")
            make_identity(nc, ident[:])

            # persistent per-head-pair K / V
            kT = [constp.tile([128, S], BF16, tag=f"kT{hp}",
                              name=f"kT{hp}") for hp in range(NHP)]
            vaug = [constp.tile([128, NSB, 130], BF16, tag=f"vaug{hp}",
                                name=f"vaug{hp}") for hp in range(NHP)]
            for hp in range(NHP):
                nc.gpsimd.memset(vaug[hp][:, :, 64], 1.0)
                nc.gpsimd.memset(vaug[hp][:, :, 129], 1.0)

            qT_store = {}

            # --- projection emitters --------------------------------------
            def rope_into(dst_ap, ps, jsl):
                qsb = ropep.tile([128, SQT], BF16, tag="qsb")
                nc.vector.tensor_copy(qsb[:], ps[:])
                t1 = ropep.tile([128, SQT], BF16, tag="t1")
                nc.vector.tensor_tensor(out=t1[:], in0=qsb[:],
                                        in1=cos_sb[:, jsl], op=mult)
                sh = ropep.tile([128, SQT], BF16, tag="sh")
                nc.vector.stream_shuffle(sh[:], qsb[:], SWAP16)
                t2 = ropep.tile([128, SQT], BF16, tag="t2")
                nc.gpsimd.tensor_tensor(out=t2[:], in0=sh[:],
                                        in1=sin_sb[:, jsl], op=mult)
                nc.vector.tensor_tensor(out=dst_ap, in0=t1[:], in1=t2[:],
                                        op=add)

            def emit_q(hp, j):
                jsl = bass.ts(j, SQT)
                ps = psM.tile([128, SQT], F32, tag="psM")
                for c in range(NC):
                    nc.tensor.matmul(ps[:], wq_sb[:, c, hp, :],
                                     xt_sb[:, c, jsl],
                                     start=(c == 0), stop=(c == NC - 1))
                    yield
                qt = qtp.tile([128, SQT], BF16, tag=f"qT{hp}")
                rope_into(qt[:], ps, jsl)
                qT_store[(hp, j)] = qt

            def emit_k(hp, j):
                jsl = bass.ts(j, SQT)
                ps = psM.tile([128, SQT], F32, tag="psM")
                for c in range(NC):
                    nc.tensor.matmul(ps[:], wk_sb[:, c, hp, :],
                                     xt_sb[:, c, jsl],
                                     start=(c == 0), stop=(c == NC - 1))
                    yield
                rope_into(kT[hp][:, jsl], ps, jsl)

            def emit_v(hp, j):
                jsl = bass.ts(j, SQT)
                ps = psM.tile([128, SQT], F32, tag="psM")
                for c in range(NC):
                    nc.tensor.matmul(ps[:], wv_sb[:, c, hp, :],
                                     xt_sb[:, c, jsl],
                                     start=(c == 0), stop=(c == NC - 1))
                    yield
                vt_sb = vtp.tile([128, SQT], BF16, tag="vt")
                nc.vector.tensor_copy(vt_sb[:], ps[:])
                for t in range(SQT // 128):
                    sb = j * (SQT // 128) + t
                    tp = psM.tile([128, 128], BF16, tag="psM")
                    nc.tensor.transpose(
                        tp[:], vt_sb[:, bass.ts(t, 128)], ident[:])
                    nc.vector.tensor_copy(vaug[hp][:, sb, 0:64], tp[:, 0:64])
                    nc.vector.tensor_copy(vaug[hp][:, sb, 65:129],
                                          tp[:, 64:128])
                    yield

            # --- Wo emitter (one unit per 128-row output block) ------------
            def emit_wo(sb, t, ag0c, ag1c):
                tsl = bass.ts(t, 128)
                ps = psM.tile([128, SQT], F32, tag="psM")
                n8 = 0
                for g, agc in ((0, ag0c), (1, ag1c)):
                    for hp in range(NHP):
                        nc.tensor.matmul(
                            ps[:], agc[:, hp, tsl],
                            wo_sb[:, NHP * g + hp, :],
                            start=(n8 == 0), stop=(n8 == 7))
                        n8 += 1
                        yield
                ost = ostp.tile([128, SQT], F32, tag="ost")
                nc.vector.tensor_copy(ost[:], ps[:])
                nc.sync.dma_start(out[bass.ts(sb, 128), :], ost[:])

            # --- filler machinery: generators yielding per-matmul ---------
            # Interleaves projection/Wo matmuls one at a time between
            # attention blocks so the in-order PE queue always has ready
            # work while exp runs on the scalar engine.
            fillers = []          # list of [generator, approx_steps_left]
            state = {"blocks_left": 0, "err": 0.0}

            def _steps_left():
                return sum(s for _, s in fillers)

            def _advance(n):
                while n > 0 and fillers:
                    ent = fillers[0]
                    try:
                        next(ent[0])
                        ent[1] = max(ent[1] - 1, 0)
                        n -= 1
                    except StopIteration:
                        fillers.pop(0)

            def pop_fillers(extra=0):
                if extra:
                    _advance(extra)
                if not fillers:
                    return
                state["err"] += _steps_left() / max(state["blocks_left"], 1)
                k = int(state["err"])
                if k > 0:
                    state["err"] -= k
                    _advance(k)

            # --- attention for one (hp, j) --------------------------------
            def attention(hp, j):
                n = 4 * j + 4
                qt = qT_store.pop((hp, j))
                pv = psPV.tile([128, 2 * SQT], F32, tag="pv")
                qk = {}
                probs = {}

                def emit_qk(i):
                    m = i - 4 * j
                    q2 = psQK.tile([128, 2 * SQT], F32, tag="qk")
                    diag = m >= 0
                    w = SQT - 128 * max(m, 0)
                    for h in range(2):
                        nc.tensor.matmul(
                            q2[:, SQT * h:SQT * h + w],
                            kT[hp][64 * h:64 * h + 64, bass.ts(i, 128)],
                            qt[64 * h:64 * h + 64, SQT - w:SQT],
                            start=True, stop=not diag,
                            skip_group_check=diag)
                    if diag:
                        # fold the causal triangle into PSUM: adds -B to the
                        # masked half of the leading 128-wide strip, so exp
                        # yields ~0 there and no mask multiply is needed
                        for h in range(2):
                            nc.tensor.matmul(
                                q2[:, SQT * h:SQT * h + 128],
                                biasL[:], biasR[:],
                                start=False, stop=True,
                                skip_group_check=True)
                    qk[i] = q2

                def emit_exp(i):
                    m = i - 4 * j
                    pr = probsp.tile([128, 2 * SQT], BF16, tag="pr")
                    if m <= 0:
                        nc.scalar.activation(pr[:], qk[i][:], Exp, scale=0.125)
                    else:
                        w_tot = 2 * SQT - 128 * m
                        nc.scalar.activation(pr[:, 0:w_tot], qk[i][:, 0:w_tot],
                                             Exp, scale=0.125)
                    del qk[i]
                    probs[i] = pr

                def emit_pv(i):
                    m = i - 4 * j
                    first = (i == 0)
                    last = (i == n - 1)
                    pr = probs.pop(i)
                    for h in range(2):
                        vsl = vaug[hp][:, i, 65 * h:65 * h + 65]
                        if m <= 0:
                            nc.tensor.matmul(
                                pv[0:65, SQT * h:SQT * h + SQT], vsl,
                                pr[:, bass.ts(h, SQT)],
                                start=first, stop=last)
                        else:
                            w = SQT - 128 * m
                            nc.tensor.matmul(
                                pv[0:65, SQT * h + 128 * m:SQT * h + SQT],
                                vsl, pr[:, SQT * h:SQT * h + w],
                                start=False, stop=last,
                                skip_group_check=True)

                emit_qk(0)
                pop_fillers(extra=1)
                emit_qk(1)
                pop_fillers(extra=1)
                for i in range(n):
                    emit_exp(i)
                    state["blocks_left"] -= 1
                    pop_fillers()
                    if i + 2 < n:
                        emit_qk(i + 2)
                    emit_pv(i)

                # --- normalize + emit into attnT tile ---------------------
                den = normp.tile([1, 2 * SQT], F32, tag="den")
                nc.vector.tensor_copy(den[:], pv[64:65, :])
                rec = normp.tile([1, 2 * SQT], F32, tag="rec")
                nc.vector.reciprocal_approx_fast(out=rec[:], in_=den[:])
                reca = normp.tile([64, SQT], F32, tag="reca")
                nc.gpsimd.partition_broadcast(reca[:], rec[0:1, 0:SQT],
                                              channels=64)
                recb = normp.tile([64, SQT], F32, tag="recb")
                nc.gpsimd.partition_broadcast(recb[:], rec[0:1, SQT:2 * SQT],
                                              channels=64)
                nc.vector.tensor_tensor(
                    out=attnT_j[0:64, hp, :], in0=pv[0:64, 0:SQT],
                    in1=reca[:], op=mult)
                nc.vector.tensor_tensor(
                    out=attnT_j[64:128, hp, :], in0=pv[0:64, SQT:2 * SQT],
                    in1=recb[:], op=mult)

            def run_all(gen):
                for _ in gen:
                    pass

            # --- prologue: projections for j = 0 --------------------------
            for hp in range(NHP):
                run_all(emit_q(hp, 0))
                run_all(emit_k(hp, 0))
                run_all(emit_v(hp, 0))

            # --- main loop ------------------------------------------------
            for j in range(NSQ):
                # queue next tile's projections as PE filler
                if j + 1 < NSQ:
                    for hp in range(NHP):
                        fillers.append([emit_q(hp, j + 1), 9])
                        fillers.append([emit_k(hp, j + 1), 9])
                        fillers.append([emit_v(hp, j + 1), 13])

                state["blocks_left"] = NHP * (4 * j + 4)
                attnT_j = attnp.tile([128, NHP, SQT], BF16, tag="attnT")
                ag0c = agp.tile([128, NHP, SQT], BF16, tag="ag0c")
                ag1c = agp.tile([128, NHP, SQT], BF16, tag="ag1c")
                for hp in range(NHP):
                    attention(hp, j)
                    # per-head-pair AllGather so the pair's Wo work can
                    # start before the whole tile finishes
                    ag_in = dramp.tile([128, SQT], BF16, tag="ag_in")
                    nc.sync.dma_start(ag_in[:], attnT_j[:, hp, :])
                    ag_out = dramp.tile([2, 128, SQT], BF16, tag="ag_out")
                    nc.gpsimd.collective_compute(
                        "AllGather", mybir.AluOpType.bypass,
                        ins=[ag_in[:].opt()], outs=[ag_out[:].opt()],
                        replica_groups=groups)
                    nc.sync.dma_start(ag0c[:, hp, :], ag_out[0])
                    nc.sync.dma_start(ag1c[:, hp, :], ag_out[1])

                # queue Wo for this tile (popped during the next tile)
                for t in range(SQT // 128):
                    sb = (SQT // 128) * j + t
                    fillers.append([emit_wo(sb, t, ag0c, ag1c), 9])

            while fillers:
                _advance(1)

    nc.compile()
    return nc


# ---------------------------------------------------------------------------
# Host-side sharding / unsharding
# ---------------------------------------------------------------------------

def _host_inputs(x, Wq, Wk, Wv, Wo, token_positions, n_cores, S):
    import ml_dtypes
    bf16 = ml_dtypes.bfloat16
    D = D_MODEL
    NC = D // 128
    NHP = 4

    # rope tables.  Partition layout within each head (64 partitions):
    # [e0..e15, o0..o15, e16..e31, o16..o31] -- the rotation partner sits
    # 16 partitions away inside the same 32-group, so the kernel's
    # stream_shuffle (a per-32-group lane shuffle) can realize the swap.
    pos = np.asarray(token_positions).astype(np.float32)  # (S,)
    i32 = np.arange(32, dtype=np.float32)
    inv_freq = ROPE_THETA ** (-i32 / 32.0)
    ang = pos[None, :] * inv_freq[:, None]              # (32, S)
    p = np.arange(128)
    pp = p % 64
    g, o = pp // 32, pp % 32
    freq_idx = 16 * g + (o % 16)                        # (128,)
    sign = np.where(o % 32 < 16, -1.0, 1.0)             # even slots: -sin
    cosT = np.cos(ang[freq_idx, :]).astype(bf16)        # (128, S)
    sinT = (sign[:, None] * np.sin(ang[freq_idx, :])).astype(bf16)

    # triangle-bias matmul constants: (biasL.T @ biasR)[p, f] = -B iff p > f
    # (the causally-masked half of a diagonal 128-strip); exp then gives ~0
    biasL = (np.arange(128)[None, :] > np.arange(128)[:, None]).astype(bf16)
    biasR = (-400.0 * np.eye(128, dtype=np.float32)).astype(bf16)

    # de-interleaving row permutation for q/k (see rope table comment)
    def qk_rows(grp):
        rows = []
        for h in range(8 * grp, 8 * grp + 8):
            rows += [h * DH + 2 * i for i in range(16)]
            rows += [h * DH + 2 * i + 1 for i in range(16)]
            rows += [h * DH + 2 * i for i in range(16, 32)]
            rows += [h * DH + 2 * i + 1 for i in range(16, 32)]
        return rows

    def wqk_layout(W, grp):
        # (D, 512) -> [128, NC, NHP, 128]
        t = W[qk_rows(grp), :].T.astype(bf16)
        return np.ascontiguousarray(
            t.reshape(NC, 128, NHP, 128).transpose(1, 0, 2, 3))

    def wv_layout(W, grp):
        t = W[512 * grp:512 * grp + 512, :].T.astype(bf16)
        return np.ascontiguousarray(
            t.reshape(NC, 128, NHP, 128).transpose(1, 0, 2, 3))

    def wo_layout(W, grp):
        t = W.T[:, 512 * grp:512 * grp + 512].astype(bf16)  # (D, 512)
        return np.ascontiguousarray(t.reshape(NC, 128, SQT).transpose(1, 0, 2))

    in_maps = []
    for c in range(n_cores):
        b, grp = c // 2, c % 2
        xb = np.ascontiguousarray(x[b].T).astype(bf16)  # (D, S)
        in_maps.append({
            "xT": np.ascontiguousarray(
                xb.reshape(NC, 128, S).transpose(1, 0, 2)),
            "wqT": wqk_layout(Wq, grp),
            "wkT": wqk_layout(Wk, grp),
            "wvT": wv_layout(Wv, grp),
            "woT": wo_layout(Wo, grp),
            "cosT": cosT,
            "sinT": sinT,
            "biasLT": biasL,
            "biasRT": biasR,
        })
    return in_maps


def _assemble(results, n_cores, S):
    B = n_cores // 2
    full = np.empty((B, S, D_MODEL), dtype=np.float32)
    for c in range(n_cores):
        b, grp = c // 2, c % 2
        full[b, :, 512 * grp:512 * grp + 512] = results[c]["out"]
    return full


# ---------------------------------------------------------------------------
# Entry point
# ---------------------------------------------------------------------------

_NC_CACHE = {}


def _get_nc(n_cores, S):
    key = (n_cores, S)
    if key not in _NC_CACHE:
        _NC_CACHE[key] = build_kernel(n_cores, S)
    return _NC_CACHE[key]


def kernel(x, Wq, Wk, Wv, Wo, token_positions, _trace=False, _tmpdir=None):
    from concourse.bass_utils import run_bass_kernel_spmd

    x = np.asarray(x)
    B, S, D = x.shape
    n_cores = 2 * B
    nc = _get_nc(n_cores, S)
    in_maps = _host_inputs(np.asarray(x), np.asarray(Wq), np.asarray(Wk),
                           np.asarray(Wv), np.asarray(Wo),
                           np.asarray(token_positions), n_cores, S)
    res = run_bass_kernel_spmd(nc, in_maps, core_ids=list(range(n_cores)),
                               trace=_trace, tmpdir=_tmpdir)
    out = _assemble(res.results, n_cores, S)
    if _trace:
        return out, res
    return out
